# revision 11
# baseline (speedup 1.0000x reference)
"""Trainium2 Bass kernel for nn_FSAS_24953759990225.

Pipeline (per core, 8 cores = 4 batches x 2 H-halves):
  x [64,130,256]+halo --(fused 1x1conv+dw3x3 as 5 tap-pair matmuls, PE)--> q,k,v [128ch, px]
  q,k --(PE transpose)--> pixel-major --(packed-RDFT matmul)--> spectra
  --(DVE packed complex mul)--> --(inverse RDFT matmul)--> Z
  --(LayerNorm over ch, per-pixel; DVE)--> --(PE transpose back)--> gate by v --> W_out matmul --> y

All matmuls bf16 operands with f32 PSUM accumulation. Pixel ordering is
patch-major: px = 64*patch + 8*py + pxx, patches ordered (patch_row, patch_col).
"""
import numpy as np
import ml_dtypes

bf16 = ml_dtypes.bfloat16

# ---------------------------------------------------------------- constants
B, C, H, W = 4, 64, 256, 256
HC = 128            # rows per core
SLAB_H = 131        # 1 halo above + 128 + 1 halo below + 1 pad row
SLAB_W = 260        # 1 pad left + 256 + 3 pad right
N_STRIPS = 8
SH = 16             # strip height (rows)
SPX = SH * W        # pixels per strip = 4096
NCORES = 8

TAPS = [(dy, dx) for dy in (-1, 0, 1) for dx in (-1, 0, 1)]


def build_freq_sets():
    pairs, selfs, seen = [], [], set()
    for u in range(8):
        for v in range(8):
            if (u, v) in seen:
                continue
            cu, cv = (-u) % 8, (-v) % 8
            if (cu, cv) == (u, v):
                selfs.append((u, v))
            else:
                pairs.append((u, v))
                seen.add((cu, cv))
            seen.add((u, v))
    return pairs, selfs


def build_RU():
    """R [64,64] pixels->packed; U [64,64] packed->pixels.

    Slot layout (32-aligned for HW start-partition rules):
      0:30   pair-re          30: self (0,0)   31: self (0,4)
      32:62  pair-im          62: self (4,0)   63: self (4,4)
    Self-conj freqs are real; their products come straight out of the
    elementwise Q*K op and are never touched by the cross ops.
    """
    pairs, selfs = build_freq_sets()
    assert selfs == [(0, 0), (0, 4), (4, 0), (4, 4)]
    R = np.zeros((64, 64), np.float64)
    U = np.zeros((64, 64), np.float64)
    ys, xs = np.meshgrid(np.arange(8), np.arange(8), indexing="ij")
    py, px = ys.ravel(), xs.ravel()

    def cosr(u, v):
        return np.cos(2 * np.pi * (u * py + v * px) / 8.0)

    def sinr(u, v):
        return np.sin(2 * np.pi * (u * py + v * px) / 8.0)

    for i, (u, v) in enumerate(pairs):
        R[i, :] = cosr(u, v)
        U[:, i] = 2.0 * cosr(u, v) / 64.0
        R[32 + i, :] = -sinr(u, v)
        U[:, 32 + i] = -2.0 * sinr(u, v) / 64.0
    for slot, (u, v) in zip((30, 31, 62, 63), selfs):
        R[slot, :] = cosr(u, v)
        U[:, slot] = cosr(u, v) / 64.0
    return R, U


def interleave_row(slot, inst):
    """Packed slot (0..63) of instance inst (0/1) -> interleaved partition row."""
    return 2 * slot + inst


def build_fwd_lhsT(swap=False):
    """lhsT [128,128]: rhs rows = [inst0 pixels 0:64 | inst1 64:128] -> interleaved packed.
    swap=True swaps the re/im 64-row blocks of the output (for the K2 spectrum)."""
    R, _ = build_RU()
    M = np.zeros((128, 128), np.float64)
    for inst in range(2):
        for slot in range(64):
            r = (interleave_row(slot, inst) + (64 if swap else 0)) % 128
            M[inst * 64:(inst + 1) * 64, r] = R[slot, :]
    return M


def build_inv_V1V2():
    """Inverse lhsTs consuming T1 = Qh*Kh and T2 = Qh*Kh2 (Kh2 = block-swapped).
    Z = V1.T @ T1 + V2.T @ T2, Z rows = [inst0 px | inst1 px]."""
    _, U = build_RU()
    V1 = np.zeros((128, 128), np.float64)
    V2 = np.zeros((128, 128), np.float64)
    for inst in range(2):
        cols = slice(inst * 64, (inst + 1) * 64)
        for s in range(30):
            V1[2 * s + inst, cols] = U[:, s]             # +U_re (QrKr)
            V1[64 + 2 * s + inst, cols] = -U[:, s]       # -U_re (QiKi)
            V2[2 * s + inst, cols] = U[:, 32 + s]        # +U_im (QrKi)
            V2[64 + 2 * s + inst, cols] = U[:, 32 + s]   # +U_im (QiKr)
        for j in range(2):
            V1[60 + 2 * j + inst, cols] = U[:, 30 + j]   # selfA products
            V1[124 + 2 * j + inst, cols] = U[:, 62 + j]  # selfB products
    return V1, V2


# ---------------------------------------------------------------- host prep
def make_slabs(x):
    """x [4,64,256,256] f32 -> 8 slabs [64,131,260] bf16 (zero-padded halos)."""
    slabs = []
    for core in range(NCORES):
        b, half = core // 2, core % 2
        h0 = half * HC
        s = np.zeros((C, SLAB_H, SLAB_W), np.float32)
        lo, hi = max(h0 - 1, 0), min(h0 + HC + 1, H)
        s[:, (lo - (h0 - 1)):(lo - (h0 - 1)) + (hi - lo), 1:257] = x[b, :, lo:hi, :]
        slabs.append(s.astype(bf16))
    return slabs


def make_weights(w1, w_dw, w_out, ln_w, ln_b):
    """Fused conv lhsTs + transform matrices + output proj, all bf16."""
    w1 = np.asarray(w1, np.float64)
    w_dw = np.asarray(w_dw, np.float64)

    def weff_T(dy, dx):   # [64, 384] = Weff(tap).T
        return (w_dw[:, 0, dy + 1, dx + 1][:, None] * w1).T

    # pass A[i], i=0..2: taps (dy,-1)+(dy,0) from x2; pass B: (-1,+1)+(0,+1) from x3;
    # pass C: (1,+1) single from x2 top half.
    wA = np.stack([np.vstack([weff_T(dy, -1), weff_T(dy, 0)]) for dy in (-1, 0, 1)])
    wB = np.vstack([weff_T(-1, 1), weff_T(0, 1)])
    wC = weff_T(1, 1)
    wA_flat = wA.transpose(1, 0, 2).reshape(128, 3 * 384)   # [128, 1152], pass-major cols
    rmat = build_fwd_lhsT(False)
    rmatsw = build_fwd_lhsT(True)
    v1m, v2m = build_inv_V1V2()
    ident = np.eye(128)
    woutw = (np.asarray(w_out, np.float64) * np.asarray(ln_w, np.float64)[None, :]).T  # [128,64]
    woutb = (np.asarray(w_out, np.float64) * np.asarray(ln_b, np.float64)[None, :]).T
    use_b = bool(np.any(np.asarray(ln_b) != 0))
    c = lambda a: np.ascontiguousarray(a.astype(np.float32)).astype(bf16)
    return dict(wA=c(wA_flat), wB=c(wB), wC=c(wC), rmat=c(rmat), rmatsw=c(rmatsw),
                v1m=c(v1m), v2m=c(v2m),
                ident=c(ident), woutw=c(woutw), woutb=c(woutb)), use_b


# ---------------------------------------------------------------- program
_CACHE = {}


def build_program(use_b, reps=1):
    import concourse.bass as bass
    import concourse.mybir as mybir
    import concourse.tile as tile
    import concourse.bacc as bacc
    from contextlib import ExitStack

    dtb = mybir.dt.bfloat16
    dtf = mybir.dt.float32

    nc = bacc.Bacc("TRN2", target_bir_lowering=False, debug=False, num_devices=NCORES)

    xp = nc.dram_tensor("xp", [C, SLAB_H, SLAB_W], dtb, kind="ExternalInput").ap()
    d_wA = nc.dram_tensor("wA", [128, 3 * 384], dtb, kind="ExternalInput").ap()
    d_wB = nc.dram_tensor("wB", [128, 384], dtb, kind="ExternalInput").ap()
    d_wC = nc.dram_tensor("wC", [64, 384], dtb, kind="ExternalInput").ap()
    d_rmat = nc.dram_tensor("rmat", [128, 128], dtb, kind="ExternalInput").ap()
    d_rmatsw = nc.dram_tensor("rmatsw", [128, 128], dtb, kind="ExternalInput").ap()
    d_v1m = nc.dram_tensor("v1m", [128, 128], dtb, kind="ExternalInput").ap()
    d_v2m = nc.dram_tensor("v2m", [128, 128], dtb, kind="ExternalInput").ap()
    d_ident = nc.dram_tensor("ident", [128, 128], dtb, kind="ExternalInput").ap()
    d_woutw = nc.dram_tensor("woutw", [128, 64], dtb, kind="ExternalInput").ap()
    d_woutb = nc.dram_tensor("woutb", [128, 64], dtb, kind="ExternalInput").ap()
    y_d = nc.dram_tensor("y", [C, HC, W], dtf, kind="ExternalOutput").ap()

    ALU = mybir.AluOpType
    ACTF = mybir.ActivationFunctionType

    with tile.TileContext(nc) as tc, ExitStack() as ctx:
        const = ctx.enter_context(tc.tile_pool(name="const", bufs=1))
        xpool = ctx.enter_context(tc.tile_pool(name="xp", bufs=2))
        qkvp = ctx.enter_context(tc.tile_pool(name="qkv", bufs=1))
        tp = ctx.enter_context(tc.tile_pool(name="tp", bufs=1))       # transposed q,k
        fp = ctx.enter_context(tc.tile_pool(name="fp", bufs=1))       # spectra
        zp = ctx.enter_context(tc.tile_pool(name="zp", bufs=1))       # Z / Zn / ZnA / zg
        sp_ = ctx.enter_context(tc.tile_pool(name="sp", bufs=1))      # stats (f32, small)
        yp = ctx.enter_context(tc.tile_pool(name="yp", bufs=2))
        # PSUM pools
        pconv = ctx.enter_context(tc.tile_pool(name="pconv", bufs=3, space="PSUM"))
        ptr = ctx.enter_context(tc.tile_pool(name="ptr", bufs=2, space="PSUM"))
        pgen = ctx.enter_context(tc.tile_pool(name="pgen", bufs=3, space="PSUM"))

        # --- resident constants
        consts = {}
        for nm, dram, shp in [("wA", d_wA, [128, 3 * 384]), ("wB", d_wB, [128, 384]),
                              ("wC", d_wC, [64, 384]), ("rmat", d_rmat, [128, 128]),
                              ("rmatsw", d_rmatsw, [128, 128]), ("v1m", d_v1m, [128, 128]),
                              ("v2m", d_v2m, [128, 128]), ("ident", d_ident, [128, 128]),
                              ("woutw", d_woutw, [128, 64]), ("woutb", d_woutb, [128, 64])]:
            if nm == "woutb" and not use_b:
                continue
            t = const.tile(shp, dtb, tag=nm, name=nm)
            nc.sync.dma_start(t[:], dram)
            consts[nm] = t
        t_wA, t_wB, t_wC = consts["wA"], consts["wB"], consts["wC"]
        t_rmat, t_rmatsw, t_id = consts["rmat"], consts["rmatsw"], consts["ident"]
        t_v1m, t_v2m = consts["v1m"], consts["v2m"]
        t_ww, t_wb = consts["woutw"], consts.get("woutb")

        from contextlib import nullcontext
        loop_cm = tc.For_i(0, reps, 1) if reps > 1 else nullcontext()
        with loop_cm:
          for s in range(N_STRIPS):
            r0 = s * SH
            # --- load x strips: x2 = [x ; x shifted (0,+1)], x3 = [x ; x shifted (+1,0)]
            x2 = xpool.tile([128, SH + 2, SLAB_W - 1], dtb, tag="x2")
            nc.sync.dma_start(x2[0:64], xp[:, r0:r0 + SH + 2, 0:SLAB_W - 1])
            nc.sync.dma_start(x2[64:128], xp[:, r0:r0 + SH + 2, 1:SLAB_W])
            x3 = xpool.tile([128, SH + 2, SLAB_W], dtb, tag="x3")
            nc.sync.dma_start(x3[0:64], xp[:, r0:r0 + SH + 2, :])
            nc.sync.dma_start(x3[64:128], xp[:, r0 + 1:r0 + SH + 3, :])

            def conv_rhs(xt, dy, dx, g, k128):
                # output cols (patch, py, pxx) for group g: patches row g//4, cols 8*(g%4)..+8
                pr, pg = g // 4, g % 4
                row = 1 + dy + 8 * pr
                col = 1 + dx + 64 * pg
                v = xt[0:128 if k128 else 64, row:row + 8, col:col + 64]
                return v.rearrange("c h (p x) -> c p h x", p=8)

            # --- fused conv: q,k,v chunks
            qkv = [qkvp.tile([128, SPX], dtb, tag=t, name=t) for t in ("q", "k", "v")]
            for c_ in range(3):
                for g in range(8):
                    ps = pconv.tile([128, 512], dtf, tag="pconv")
                    if True:
                        out = ps[:]
                        for p in range(5):
                            if p < 3:      # pass A[p]: taps (dy,-1),(dy,0) on x2
                                dy = p - 1
                                lhsT = t_wA[:, 384 * p + 128 * c_: 384 * p + 128 * (c_ + 1)]
                                rhs = conv_rhs(x2, dy, -1, g, True)
                            elif p == 3:   # pass B: taps (-1,+1),(0,+1) on x3
                                lhsT = t_wB[:, 128 * c_:128 * (c_ + 1)]
                                rhs = conv_rhs(x3, -1, 1, g, True)
                            else:          # pass C: tap (1,+1) on x2 top half
                                lhsT = t_wC[:, 128 * c_:128 * (c_ + 1)]
                                rhs = conv_rhs(x2, 1, 1, g, False)
                            nc.tensor.matmul(out, lhsT, rhs, start=(p == 0), stop=(p == 4))
                    nc.scalar.copy(qkv[c_][:, 512 * g:512 * (g + 1)], ps[:])

            # --- transpose q,k to pixel-major [2patch x 64px, 128ch-cols]
            qT = [tp.tile([128, SPX], dtb, tag=t, name=t) for t in ("qT", "kT")]
            for c_ in range(2):
                for a4 in range(8):        # 4 blocks per psum tile
                    ps = ptr.tile([128, 512], dtb, tag="ptr")
                    for j in range(4):
                        blk = 4 * a4 + j
                        nc.tensor.transpose(
                            ps[:, 128 * j:128 * (j + 1)],
                            qkv[c_][:, 128 * blk:128 * (blk + 1)], t_id[:])
                    nc.scalar.copy(qT[c_][:, 512 * a4:512 * (a4 + 1)], ps[:])

            # --- forward packed RDFT: Qh, Kh (normal), Kh2 (block-swapped)
            QK = [fp.tile([128, SPX], dtb, tag=t, name=t) for t in ("Qh", "Kh", "Kh2")]
            for c_, mat, dst in ((0, t_rmat, QK[0]), (1, t_rmat, QK[1]), (1, t_rmatsw, QK[2])):
                for g in range(8):
                    ps = pgen.tile([128, 512], dtf, tag="pgen")
                    nc.tensor.matmul(ps[:], mat[:],
                                     qT[c_][:, 512 * g:512 * (g + 1)],
                                     start=True, stop=True)
                    nc.scalar.copy(dst[:, 512 * g:512 * (g + 1)], ps[:])

            # --- packed complex multiply: T1 = Qh*Kh, T2 = Qh*Kh2
            Qh, Kh, Kh2 = QK
            T1 = fp.tile([128, SPX], dtb, tag="T1")
            T2 = fp.tile([128, SPX], dtb, tag="T2")
            nc.vector.tensor_tensor(T1[:], Qh[:], Kh[:], ALU.mult)
            nc.vector.tensor_tensor(T2[:], Qh[:], Kh2[:], ALU.mult)

            # --- inverse RDFT -> Z [2patch x 64px rows, 128 ch cols per block]
            Z = zp.tile([128, SPX], dtb, tag="Z")
            for g in range(8):
                ps = pgen.tile([128, 512], dtf, tag="pgen")
                nc.tensor.matmul(ps[:], t_v1m[:], T1[:, 512 * g:512 * (g + 1)],
                                 start=True, stop=False)
                nc.tensor.matmul(ps[:], t_v2m[:], T2[:, 512 * g:512 * (g + 1)],
                                 start=False, stop=True)
                nc.scalar.copy(Z[:, 512 * g:512 * (g + 1)], ps[:])

            # --- LayerNorm stats over 128-ch groups (per pixel-row x patch-block)
            nblk = SPX // 128   # 32
            sum_t = sp_.tile([128, nblk], dtf, tag="sum")
            ssq_t = sp_.tile([128, nblk], dtf, tag="ssq")
            zsq = fp.tile([128, SPX], dtb, tag="ctmp2")
            Zv = Z[:].rearrange("c (b k) -> c b k", k=128)
            nc.vector.tensor_reduce(sum_t[:], Zv, mybir.AxisListType.X, ALU.add)
            nc.scalar.activation(zsq[:], Z[:], ACTF.Square)
            nc.vector.tensor_reduce(ssq_t[:], zsq[:].rearrange("c (b k) -> c b k", k=128),
                                    mybir.AxisListType.X, ALU.add)
            mu = sp_.tile([128, nblk], dtf, tag="mu")
            var = sp_.tile([128, nblk], dtf, tag="var")
            rstd = sp_.tile([128, nblk], dtf, tag="rstd")
            nc.vector.tensor_scalar_mul(mu[:], sum_t[:], 1.0 / 128.0)
            nc.vector.tensor_tensor(var[:], mu[:], mu[:], ALU.mult)
            # var = ssq/128 - mu^2 + eps  ==  (ssq * 1/128) - (var - eps)... do in 2 ops:
            nc.vector.scalar_tensor_tensor(var[:], ssq_t[:], 1.0 / 128.0, var[:],
                                           ALU.mult, ALU.subtract)
            nc.vector.tensor_scalar_add(var[:], var[:], 1e-5)
            nc.vector.reciprocal(rstd[:], var[:])
            nc.scalar.sqrt(rstd[:], rstd[:])
            # --- normalize per block: Zn = (Z - mu)*rstd  (single-src tensor_scalar, 4x)
            Zn = zp.tile([128, SPX], dtb, tag="Zn")
            for b_ in range(nblk):
                nc.vector.tensor_scalar(
                    Zn[:, 128 * b_:128 * (b_ + 1)], Z[:, 128 * b_:128 * (b_ + 1)],
                    mu[:, b_:b_ + 1], rstd[:, b_:b_ + 1], ALU.subtract, ALU.mult)

            # --- transpose back to [128ch, px]
            ZnA = zp.tile([128, SPX], dtb, tag="ZnA")
            for a4 in range(8):
                ps = ptr.tile([128, 512], dtb, tag="ptr")
                for j in range(4):
                    blk = 4 * a4 + j
                    nc.tensor.transpose(
                        ps[:, 128 * j:128 * (j + 1)],
                        Zn[:, 128 * blk:128 * (blk + 1)], t_id[:])
                nc.scalar.copy(ZnA[:, 512 * a4:512 * (a4 + 1)], ps[:])

            # --- gate by v
            zg = zp.tile([128, SPX], dtb, tag="zg")
            nc.vector.tensor_tensor(zg[:], ZnA[:], qkv[2][:], ALU.mult)

            # --- W_out projection + unpatch drain
            ysb = yp.tile([64, SH, W], dtf, tag="ysb")
            for g in range(8):
                ps = pgen.tile([64, 512], dtf, tag="pgen")
                nc.tensor.matmul(ps[:], t_ww[:], zg[:, 512 * g:512 * (g + 1)],
                                 start=True, stop=not use_b)
                if use_b:
                    nc.tensor.matmul(ps[:], t_wb[:], qkv[2][:, 512 * g:512 * (g + 1)],
                                     start=False, stop=True)
                pr, pg = g // 4, g % 4
                dest = ysb[:, 8 * pr:8 * pr + 8, 64 * pg:64 * pg + 64]
                dest = dest.rearrange("c h (p x) -> c p h x", p=8)
                nc.scalar.copy(dest, ps[:])
            nc.sync.dma_start(y_d[:, r0:r0 + SH, :], ysb[:])

    nc.compile()
    return nc


def get_program(use_b, reps=1):
    key = ("prog", use_b, reps)
    if key not in _CACHE:
        _CACHE[key] = build_program(use_b, reps)
    return _CACHE[key]


# ---------------------------------------------------------------- entry point
def kernel(x, w1, w_dw, w_out, ln_w, ln_b):
    from concourse.bass_utils import run_bass_kernel_spmd

    x = np.asarray(x)
    wd, use_b = make_weights(w1, w_dw, w_out, ln_w, ln_b)
    slabs = make_slabs(np.asarray(x, np.float32))
    nc = get_program(use_b)

    in_maps = []
    for core in range(NCORES):
        m = dict(xp=slabs[core], **wd)
        in_maps.append(m)
    res = run_bass_kernel_spmd(nc, in_maps, core_ids=list(range(NCORES)))
    y = np.empty((B, C, H, W), np.float32)
    for core in range(NCORES):
        b, half = core // 2, core % 2
        y[b, :, half * HC:(half + 1) * HC, :] = res.results[core]["y"]
    return y
